# revision 2
# baseline (speedup 1.0000x reference)
"""GCN (2x GCNConv + linear head) on 8 Trainium2 NeuronCores — v2.

Design notes (from microbenchmarking this backend):
  - Each instruction has a large fixed cost (~30-110us) with small per-byte
    cost (DMA ~75GB/s marginal, DVE ~free up to MBs, ap_gather ~30ns/idx,
    matmul N=512 ~10us). So: few, fat instructions win.
  - dst-sharded across 8 cores; feature-major [128 feat, node] layout
    everywhere; 128x128 weights replicated.
  - Per layer: stage A computes the shard's scaled table t = dinv*(h@W) via
    25 wide matmuls; one AllGather shares the bf16 table; aggregation runs
    8 windows (one per src core): SWDGE cast-load (bf16->f32), one/two big
    ap_gather calls into a degree-class grid, ~10 segmented tensor_reduces,
    one canonical-order ap_gather, and a DMA-accumulate (CCE add) into a
    DRAM f32 accumulator. dinv_dst/bias/relu applied once per layer.
  - Grid structure (class sizes, gather counts) must be identical across
    cores (single SPMD program): sizes are maxed over cores and padded with
    a guaranteed-zero window column.
"""

import numpy as np
import ml_dtypes

import concourse.bacc as bacc
import concourse.mybir as mybir
import concourse.tile as tile
from concourse import library_config
from concourse.bass_utils import run_bass_kernel_spmd

N_NODES = 100000
N_CORES = 8
SHARD = 12500
PS = 12544           # padded shard size (98 * 128)
D = 128
P = 128
ZCOL = 12540         # guaranteed-zero column in every window (pad target)
# degree classes: (lo, width) — dsts with lo <= count <= width, padded to
# width columns; counts above the last class go to a max-width tail
CLASS_DEFS = [(1, 1), (2, 2), (3, 3), (4, 4), (5, 5), (6, 8)]
CCAP = 8             # counts above this go to the tail class
GPIECE = 13184       # max grid-gather slots per call (51.5KB/partition f32)

F32 = mybir.dt.float32
BF16 = mybir.dt.bfloat16
I16 = mybir.dt.int16
AX = mybir.AxisListType
ALU = mybir.AluOpType
ACTF = mybir.ActivationFunctionType


def _pack_idx(flat):
    """flat int array -> [128, ceil(n/128)*8] int16 wrapped layout.

    Pads to a multiple of 128 indices (8 idx-tile columns) so that every
    packed block starts 16-byte aligned — the gpsimd ucode reads indices
    with wide loads and silently corrupts on misaligned bases.
    """
    flat = np.asarray(flat, np.int64)
    pad = (-len(flat)) % 128
    if pad:
        flat = np.concatenate([flat, np.full(pad, ZCOL, np.int64)])
    wrapped = flat.reshape(-1, 16).T.astype(np.int16)
    return np.tile(wrapped, (8, 1))


def _plan(edge_index):
    src = np.asarray(edge_index[0], np.int64)
    dst = np.asarray(edge_index[1], np.int64)
    # self-loops are handled separately on-device (own-shard table add);
    # only real edges go through the gather path. deg still counts the loop.
    ms = src
    md = dst

    deg = np.bincount(md, minlength=N_NODES).astype(np.float64) + 1.0
    dinv = (1.0 / np.sqrt(deg)).astype(np.float32)

    k = md // SHARD
    w = ms // SHARD
    dl = md - k * SHARD
    sl = ms - w * SHARD

    order = np.argsort((k * N_CORES + w) * PS + dl, kind="stable")
    k_s, w_s, dl_s, sl_s = k[order], w[order], dl[order], sl[order]
    blk = k_s * N_CORES + w_s
    blk_start = np.searchsorted(blk, np.arange(64))
    blk_end = np.searchsorted(blk, np.arange(64), side="right")

    # pass 1: per (k, w) class histograms -> cross-core maxima per w
    cnts = np.zeros((N_CORES, N_CORES, PS), np.int32)
    for kk in range(N_CORES):
        for ww in range(N_CORES):
            s, e = blk_start[kk * 8 + ww], blk_end[kk * 8 + ww]
            cnts[kk, ww] = np.bincount(dl_s[s:e], minlength=PS)
    nclasses = len(CLASS_DEFS)
    n_c = np.zeros((N_CORES, N_CORES, nclasses), np.int64)  # [k, w, ci]
    for ci, (lo, wid) in enumerate(CLASS_DEFS):
        n_c[:, :, ci] = ((cnts >= lo) & (cnts <= wid)).sum(axis=2)
    n_tail = (cnts > CCAP).sum(axis=2)                      # [k, w]
    W_tail = cnts.max(axis=2)                               # [k, w]

    N_c = n_c.max(axis=0)          # [w, ci] uniform class sizes
    N_t = n_tail.max(axis=0)       # [w]
    W_t = W_tail.max(axis=0)       # [w]  (tail width; may be <= CCAP if no tail)

    # per-w uniform structure
    # grid layout: class blocks (CLASS_DEFS order) then tail block
    # wpart layout: col 0 = zero, then class blocks, then tail block
    plans = []                     # per w: dict(classes=[(w, n, goff, woff)], G, WPW, pieces)
    for ww in range(N_CORES):
        classes = []
        goff = 0
        woff = 1
        for ci, (lo, wid) in enumerate(CLASS_DEFS):
            n = int(N_c[ww, ci])
            if n:
                classes.append((wid, n, goff, woff))
                goff += wid * n
                woff += n
        nt, wt = int(N_t[ww]), int(W_t[ww])
        if nt and wt > CCAP:
            classes.append((wt, nt, goff, woff))
            goff += wt * nt
            woff += nt
        # split into gather pieces <= GPIECE at row boundaries
        pieces = []                # (gstart, gsize, [(c, n, goff_rel, woff)])
        cur = []
        pstart = 0
        for (c, n, go, wo) in classes:
            off = 0
            while off < n:
                space = GPIECE - (go + off * c - pstart)
                take = min(n - off, space // c)
                if take <= 0:
                    gsz = go + off * c - pstart
                    pieces.append((pstart, gsz, cur))
                    cur = []
                    pstart = go + off * c
                    take = min(n - off, GPIECE // c)
                cur.append((c, take, go + off * c, wo + off))
                off += take
        if cur:
            pieces.append((pstart, goff - pstart, cur))
        plans.append(dict(G=goff, WPW=woff, pieces=pieces))
    WPMAX = max(pl["WPW"] for pl in plans)

    # pass 2: per-core idx arrays
    idx_cols_per_w = []
    for ww in range(N_CORES):
        cols = sum((gsz + 127) // 128 * 8
                   for (_, gsz, _) in plans[ww]["pieces"])
        cols += PS // 16
        idx_cols_per_w.append(cols)
    IDXC = sum(idx_cols_per_w)

    idx_all = np.empty((N_CORES, P, IDXC), np.int16)
    gridpad = np.zeros(N_CORES, np.int64)
    for kk in range(N_CORES):
        parts = []
        for ww in range(N_CORES):
            s, e = blk_start[kk * 8 + ww], blk_end[kk * 8 + ww]
            cnt = cnts[kk, ww]
            starts = np.zeros(PS + 1, np.int64)
            np.cumsum(cnt, out=starts[1:])
            starts += s
            pl = plans[ww]
            grid = np.full(pl["G"], ZCOL, np.int64)
            pos = np.zeros(PS, np.int64)    # wpart position per dst (0=absent)

            def fill_class(dlist, wid, gbase, wbase):
                if not len(dlist):
                    return
                posm = starts[dlist][:, None] + np.arange(wid)[None, :]
                valid = np.arange(wid)[None, :] < cnt[dlist][:, None]
                vals = np.where(valid, sl_s[np.minimum(posm,
                                starts[dlist + 1][:, None] - 1)], ZCOL)
                grid[gbase:gbase + len(dlist) * wid] = vals.ravel()
                pos[dlist] = wbase + np.arange(len(dlist))

            gbase = 0
            wbase = 1
            for ci, (lo, wid) in enumerate(CLASS_DEFS):
                n_u = int(N_c[ww, ci])
                if not n_u:
                    continue
                fill_class(np.where((cnt >= lo) & (cnt <= wid))[0], wid,
                           gbase, wbase)
                gbase += wid * n_u
                wbase += n_u
            nt, wt = int(N_t[ww]), int(W_t[ww])
            if nt and wt > CCAP:
                fill_class(np.where(cnt > CCAP)[0], wt, gbase, wbase)
            gridpad[kk] += pl["G"] - (e - s)
            canon = np.where(cnt[:PS] > 0, pos, 0)
            for (pstart, gsz, _) in pl["pieces"]:
                parts.append(_pack_idx(grid[pstart:pstart + gsz]))
            parts.append(_pack_idx(canon))
        idx_all[kk] = np.concatenate(parts, axis=1)

    plan = dict(plans=plans, WPMAX=WPMAX, IDXC=IDXC, dinv=dinv,
                idx_cols_per_w=idx_cols_per_w, gridpad=gridpad)
    return plan, idx_all


def _build(plan, repeat=1, mode="full"):
    # mode: bisect knob — "sa" (stage A+AG), "win" (+cast loads),
    # "grid" (+gathers), "red" (+reduces), "canon" (+canon gather),
    # "accum" (+accum dma), "full"
    LVL = 6 if mode == "debug" else \
        ["sa", "win", "grid", "red", "canon", "accum", "full"].index(mode)
    IDXC = plan["IDXC"]
    WPMAX = plan["WPMAX"]
    plans = plan["plans"]

    nc = bacc.Bacc("TRN2", target_bir_lowering=False, debug=False,
                   enable_asserts=False, num_devices=N_CORES,
                   num_swdge_queues=4)

    xf_t = nc.dram_tensor("x_f", [P, PS], F32, kind="ExternalInput")
    dv_t = nc.dram_tensor("dinv_b", [P, PS], F32, kind="ExternalInput")
    idx_t = nc.dram_tensor("idx", [P, IDXC], I16, kind="ExternalInput")
    w1_t = nc.dram_tensor("W1", [D, D], F32, kind="ExternalInput")
    w2_t = nc.dram_tensor("W2", [D, D], F32, kind="ExternalInput")
    wf_t = nc.dram_tensor("Wf", [D, 1], F32, kind="ExternalInput")
    b1_t = nc.dram_tensor("b1", [P, 1], F32, kind="ExternalInput")
    b2_t = nc.dram_tensor("b2", [P, 1], F32, kind="ExternalInput")
    bf_t = nc.dram_tensor("bf", [1, 1], F32, kind="ExternalInput")
    out_t = nc.dram_tensor("out", [1, PS], F32, kind="ExternalOutput")
    debug = mode == "debug"
    if debug:
        dbg_win = nc.dram_tensor("dbg_win", [P, PS], F32,
                                 kind="ExternalOutput")
        dbg_canon = nc.dram_tensor("dbg_canon", [P, PS], F32,
                                   kind="ExternalOutput")
        dbg_acc = nc.dram_tensor("dbg_acc", [P, PS], F32,
                                 kind="ExternalOutput")
        dbg_part = nc.dram_tensor("dbg_part", [N_CORES * P, PS], F32,
                                  kind="ExternalOutput")
        dbg_wp = nc.dram_tensor("dbg_wp", [P, WPMAX], F32,
                                kind="ExternalOutput")
        dbg_grid = nc.dram_tensor("dbg_grid", [P, GPIECE], F32,
                                  kind="ExternalOutput")

    rg = [list(range(N_CORES))]

    with tile.TileContext(nc) as tc:
        with (
            tc.tile_pool(name="dram", bufs=1, space="DRAM") as dpool,
            tc.tile_pool(name="const", bufs=1) as cpool,
            tc.tile_pool(name="sb", bufs=1) as spool,
            tc.tile_pool(name="ps", bufs=2, space="PSUM") as ppool,
        ):
            nc.gpsimd.load_library(library_config.mlp)

            part_d = dpool.tile([N_CORES * P, PS], F32, name="part_d")

            # ---- constants ----
            w1s = cpool.tile([D, D], F32)
            nc.sync.dma_start(out=w1s[:], in_=w1_t.ap())
            w2s = cpool.tile([D, D], F32)
            nc.sync.dma_start(out=w2s[:], in_=w2_t.ap())
            wfs = cpool.tile([D, 1], F32)
            nc.sync.dma_start(out=wfs[:], in_=wf_t.ap())
            b1s = cpool.tile([P, 1], F32)
            nc.sync.dma_start(out=b1s[:], in_=b1_t.ap())
            b2s = cpool.tile([P, 1], F32)
            nc.sync.dma_start(out=b2s[:], in_=b2_t.ap())
            bfs = cpool.tile([1, 1], F32)
            nc.sync.dma_start(out=bfs[:], in_=bf_t.ap())
            idx_s = cpool.tile([P, IDXC], I16)
            nc.sync.dma_start(out=idx_s[:], in_=idx_t.ap())
            wpart = cpool.tile([P, WPMAX], F32)
            nc.vector.memset(wpart[:, 0:1], 0.0)

            # idx column offsets per (w): pieces then canon
            idx_off = []
            off = 0
            for ww in range(N_CORES):
                po = []
                for (_, gsz, _) in plans[ww]["pieces"]:
                    po.append(off)
                    off += (gsz + 127) // 128 * 8
                idx_off.append((po, off))
                off += PS // 16

            def stage_a(l, rhs, ag_in, table):
                """table shard = rhs @ W (f-major) -> bf16 -> ag_in -> AG."""
                wmat = w1s if l == 0 else w2s
                stg = spool.tile([P, PS], BF16, tag="buf")
                for c0 in range(0, PS, 2048):
                    csz = min(2048, PS - c0)
                    pm = ppool.tile([P, csz], F32, tag="pm", bufs=1)
                    for s0 in range(0, csz, 512):
                        ssz = min(512, csz - s0)
                        nc.tensor.matmul(pm[:, s0:s0 + ssz], lhsT=wmat[:],
                                         rhs=rhs[:, c0 + s0:c0 + s0 + ssz],
                                         start=True, stop=True)
                    nc.vector.tensor_copy(stg[:, c0:c0 + csz], pm[:])
                nc.sync.dma_start(out=ag_in[:], in_=stg[:])
                nc.gpsimd.collective_compute(
                    "AllGather", ALU.bypass, replica_groups=rg,
                    ins=[ag_in[:]], outs=[table[:]])

            def aggregate(table, dbg=False):
                for ww in range(N_CORES):
                    pl = plans[ww]
                    if LVL < 1:
                        continue
                    win = spool.tile([P, PS], F32, tag="win")
                    nc.gpsimd.dma_start(
                        out=win[:], in_=table[ww * P:(ww + 1) * P, :])
                    if dbg and ww == 1:
                        nc.sync.dma_start(out=dbg_win.ap(), in_=win[:])
                    if LVL < 2:
                        continue
                    for pi, (pstart, gsz, segs) in enumerate(pl["pieces"]):
                        g16 = (gsz + 127) // 128 * 128
                        grid = spool.tile([P, g16], F32, tag="buf")
                        nc.gpsimd.ap_gather(
                            grid[:, :, None], win[:, :, None],
                            idx_s[:, idx_off[ww][0][pi]:
                                  idx_off[ww][0][pi] + g16 // 16],
                            P, PS, 1, g16)
                        if dbg and ww == 1 and pi == 0:
                            nc.sync.dma_start(out=dbg_grid.ap()[:, :g16],
                                              in_=grid[:])
                        if LVL < 3:
                            continue
                        for (c, n, go, wo) in segs:
                            nc.vector.tensor_reduce(
                                out=wpart[:, wo:wo + n, None],
                                in_=grid[:, go - pstart:go - pstart + n * c]
                                    .rearrange("p (n c) -> p n c", c=c),
                                axis=AX.X, op=ALU.add)
                    if LVL < 4:
                        continue
                    canon = spool.tile([P, PS], F32, tag="buf")
                    nc.gpsimd.ap_gather(
                        canon[:, :, None], wpart[:, :pl["WPW"], None],
                        idx_s[:, idx_off[ww][1]:idx_off[ww][1] + PS // 16],
                        P, pl["WPW"], 1, PS)
                    if dbg and ww == 1:
                        nc.sync.dma_start(out=dbg_wp.ap(), in_=wpart[:])
                    if dbg and ww == 0:
                        nc.sync.dma_start(out=dbg_canon.ap(), in_=canon[:])
                    if LVL < 5:
                        continue
                    nc.sync.dma_start(
                        out=part_d[ww * P:(ww + 1) * P, :], in_=canon[:])

            for rep in range(repeat):
                ag_in = [dpool.tile([P, PS], BF16, name=f"agin{l}_{rep}")
                         for l in range(2)]
                table = [dpool.tile([N_CORES * P, PS], BF16,
                                    name=f"table{l}_{rep}",
                                    addr_space="Shared") for l in range(2)]
                # ================= layer 1 =================
                xf = spool.tile([P, PS], F32, tag="win")
                nc.sync.dma_start(out=xf[:], in_=xf_t.ap())
                stage_a(0, xf, ag_in[0], table[0])
                aggregate(table[0], dbg=debug)
                if debug:
                    nc.sync.dma_start(out=dbg_part.ap(), in_=part_d[:])
                accs = spool.tile([P, PS], F32, tag="win")
                nc.sync.dma_start(out=accs[:], in_=part_d[0:P, :])
                for ww in range(1, N_CORES):
                    tmp = spool.tile([P, PS], F32, tag="buf")
                    nc.sync.dma_start(out=tmp[:],
                                      in_=part_d[ww * P:(ww + 1) * P, :])
                    nc.vector.tensor_tensor(out=accs[:], in0=accs[:],
                                            in1=tmp[:], op=ALU.add)
                selft = spool.tile([P, PS], F32, tag="buf")
                nc.gpsimd.dma_start(out=selft[:], in_=ag_in[0][:])
                nc.vector.tensor_tensor(out=accs[:], in0=accs[:],
                                        in1=selft[:], op=ALU.add)
                if debug:
                    nc.sync.dma_start(out=dbg_acc.ap(), in_=accs[:])
                dvt = spool.tile([P, PS], F32, tag="buf")
                nc.sync.dma_start(out=dvt[:], in_=dv_t.ap())
                nc.vector.tensor_tensor(out=accs[:], in0=accs[:],
                                        in1=dvt[:], op=ALU.mult)
                nc.vector.tensor_scalar_add(accs[:], accs[:], b1s[:])
                nc.scalar.activation(accs[:], accs[:], ACTF.Relu)
                nc.vector.tensor_tensor(out=accs[:], in0=accs[:],
                                        in1=dvt[:], op=ALU.mult)

                # ================= layer 2 =================
                stage_a(1, accs, ag_in[1], table[1])
                aggregate(table[1])
                accs2 = spool.tile([P, PS], F32, tag="win")
                nc.sync.dma_start(out=accs2[:], in_=part_d[0:P, :])
                for ww in range(1, N_CORES):
                    tmp2 = spool.tile([P, PS], F32, tag="buf")
                    nc.sync.dma_start(out=tmp2[:],
                                      in_=part_d[ww * P:(ww + 1) * P, :])
                    nc.vector.tensor_tensor(out=accs2[:], in0=accs2[:],
                                            in1=tmp2[:], op=ALU.add)
                selft2 = spool.tile([P, PS], F32, tag="buf")
                nc.gpsimd.dma_start(out=selft2[:], in_=ag_in[1][:])
                nc.vector.tensor_tensor(out=accs2[:], in0=accs2[:],
                                        in1=selft2[:], op=ALU.add)
                dvt2 = spool.tile([P, PS], F32, tag="buf")
                nc.sync.dma_start(out=dvt2[:], in_=dv_t.ap())
                nc.vector.tensor_tensor(out=accs2[:], in0=accs2[:],
                                        in1=dvt2[:], op=ALU.mult)
                nc.vector.tensor_scalar_add(accs2[:], accs2[:], b2s[:])
                nc.scalar.activation(accs2[:], accs2[:], ACTF.Relu)

                # ================= head =================
                outst = spool.tile([1, PS], F32, tag="buf")
                for c0 in range(0, PS, 2048):
                    csz = min(2048, PS - c0)
                    pm = ppool.tile([1, csz], F32, tag="pmh", bufs=1)
                    for s0 in range(0, csz, 512):
                        ssz = min(512, csz - s0)
                        nc.tensor.matmul(pm[:, s0:s0 + ssz], lhsT=wfs[:],
                                         rhs=accs2[:, c0 + s0:c0 + s0 + ssz],
                                         start=True, stop=True)
                    nc.vector.tensor_scalar_add(outst[:, c0:c0 + csz], pm[:],
                                                bfs[:])
                nc.sync.dma_start(out=out_t.ap(), in_=outst[:])

    nc.compile()
    return nc


_CACHE: dict = {}
_PLAN_CACHE: dict = {}


def _plan_cached(edge_index):
    import hashlib
    h = hashlib.sha256(np.ascontiguousarray(edge_index).tobytes()).hexdigest()
    if h not in _PLAN_CACHE:
        _PLAN_CACHE[h] = _plan(edge_index)
    return _PLAN_CACHE[h]


def kernel(x, W1, b1, W2, b2, Wf, bf, edge_index, _trace=False, _repeat=1,
           _mode="full"):
    plan, idx_all = _plan_cached(edge_index)
    dinv = plan["dinv"]

    x = np.asarray(x, np.float32)
    common = {
        "W1": np.asarray(W1, np.float32).reshape(D, D),
        "W2": np.asarray(W2, np.float32).reshape(D, D),
        "Wf": np.asarray(Wf, np.float32).reshape(D, 1),
        "bf": np.asarray(bf, np.float32).reshape(1, 1),
        "b1": np.asarray(b1, np.float32).reshape(D, 1),
        "b2": np.asarray(b2, np.float32).reshape(D, 1),
    }
    in_maps = []
    for k in range(N_CORES):
        lo = k * SHARD
        xs = np.zeros((P, PS), np.float32)
        xs[:, :SHARD] = (x[lo:lo + SHARD] * dinv[lo:lo + SHARD, None]).T
        dvb = np.zeros((P, PS), np.float32)
        dvb[:, :SHARD] = dinv[lo:lo + SHARD][None, :]
        m = dict(common)
        m["x_f"] = xs
        m["dinv_b"] = dvb
        m["idx"] = np.ascontiguousarray(idx_all[k])
        in_maps.append(m)

    key = (plan["IDXC"], _repeat, _mode)
    if key not in _CACHE:
        _CACHE[key] = _build(plan, repeat=_repeat, mode=_mode)
    nc = _CACHE[key]
    res = run_bass_kernel_spmd(nc, in_maps, core_ids=list(range(N_CORES)),
                               trace=_trace)
    out = np.empty(N_NODES, np.float32)
    for k in range(N_CORES):
        out[k * SHARD:(k + 1) * SHARD] = res.results[k]["out"][0, :SHARD]
    kernel.last_exec_time_ns = res.exec_time_ns
    kernel.last_results = res
    return out


kernel.last_exec_time_ns = None
kernel.last_results = None
